# revision 6
# baseline (speedup 1.0000x reference)
"""Trainium2 Bass kernel for nn_Conv1dSubsampling (pointwise conv 80->512 +
depthwise conv k=8 stride=4, both with per-out-channel symmetric fake-quant).

Strategy:
  - Host: fake-quantize w1/w2, fuse into per-tap weights
      W[j, c, o] = w1q[o, c] * w2q[o, j]   (j = 0..7 tap, c in-ch, o out-ch)
    so that  out[b, o, u] = sum_j sum_c W[j, c, o] * x[b, c, 4u + j].
  - Device (8 cores, data-parallel over batch, 2 batches/core):
    for each output tile [128 out-ch, <=512 u]: accumulate 8 matmuls
    (K=80 contraction over in-ch, one per tap) into one PSUM bank.
    The per-tap rhs is a stride-4 view of the x tile (no im2col copy).
    Matmuls run in float32r (full PE rate); accumulation is fp32 in PSUM.
  - x_len_out = floor(x_len / 4) is computed on host (16 scalars).
"""

import numpy as np

IN_CH, OUT_CH, SF = 80, 512, 4
T_IN = 4096
BATCH = 16
N_CORES = 8
BPC = BATCH // N_CORES          # batches per core
KT = 2 * SF                     # depthwise taps
T_OUT = (T_IN - KT) // SF + 1   # 1023
QMAX = 2.0 ** 7 - 1.0
EPS = 1e-5

_MODULE_CACHE = {}
LAST_RESULTS = None  # BassKernelResults of the most recent run (for test.py)


def _fake_quant(w):
    """Match reference.fake_quant_weight: symmetric per-out-channel max quant."""
    w = w.astype(np.float32)
    s = np.max(np.abs(w.reshape(w.shape[0], -1)), axis=1) / np.float32(QMAX)
    s = np.maximum(s, np.float32(EPS)).astype(np.float32)
    sb = s.reshape((-1,) + (1,) * (w.ndim - 1))
    return (np.round(w / sb) * sb).astype(np.float32)


def _build_module():
    import concourse.bass as bass
    import concourse.tile as tile
    from concourse import bacc, mybir

    f32 = mybir.dt.float32
    f32r = mybir.dt.float32r

    nc = bacc.Bacc(
        "TRN2",
        target_bir_lowering=False,
        debug=False,
        enable_asserts=False,
        num_devices=N_CORES,
    )
    xs_d = nc.dram_tensor("xs", [BPC, IN_CH, T_IN], f32r, kind="ExternalInput").ap()
    wt_d = nc.dram_tensor("wt", [IN_CH, KT, OUT_CH], f32r, kind="ExternalInput").ap()
    out_d = nc.dram_tensor("out", [BPC, OUT_CH, T_OUT], f32, kind="ExternalOutput").ap()

    M_TILES = OUT_CH // 128     # 4
    U_BLK = 512

    with tile.TileContext(nc) as tc:
        with (
            tc.tile_pool(name="wp", bufs=1) as wp,
            tc.tile_pool(name="xp", bufs=2) as xp,
            tc.tile_pool(name="op", bufs=3) as op,
            tc.tile_pool(name="ps", bufs=4, space=bass.MemorySpace.PSUM) as ps,
        ):
            w_sb = wp.tile([IN_CH, KT, OUT_CH], f32r)
            nc.sync.dma_start(out=w_sb, in_=wt_d)

            for b in range(BPC):
                x_sb = xp.tile([IN_CH, T_IN], f32r)
                nc.sync.dma_start(out=x_sb, in_=xs_d[b])
                xv = x_sb.rearrange("c (u r) -> c u r", r=SF)  # [80, 1024, 4]

                for m in range(M_TILES):
                    o_sb = op.tile([128, T_OUT], f32)
                    # fp32r matmuls need an even free dim; cover 1023 outputs
                    # with two overlapping 512-wide blocks (u=511 done twice).
                    for u0 in (0, T_OUT - U_BLK):
                        n = U_BLK
                        acc = ps.tile([128, U_BLK], f32)
                        for j in range(KT):
                            # rhs: x[b, c, 4u + j] for u in [u0, u0+n)
                            q, r = divmod(j, SF)
                            rhs = xv[:, u0 + q:u0 + q + n, r]
                            nc.tensor.matmul(
                                acc[:, :n],
                                w_sb[:, j, m * 128:(m + 1) * 128],
                                rhs,
                                start=(j == 0),
                                stop=(j == KT - 1),
                            )
                        # Spread PSUM->SBUF copies over DVE and ACT
                        if m % 2 == 0:
                            nc.vector.tensor_copy(o_sb[:, u0:u0 + n], acc[:, :n])
                        else:
                            nc.scalar.copy(o_sb[:, u0:u0 + n], acc[:, :n])
                    nc.sync.dma_start(
                        out=out_d[b, m * 128:(m + 1) * 128, :], in_=o_sb
                    )
    nc.compile()
    return nc


def _get_module():
    if "nc" not in _MODULE_CACHE:
        _MODULE_CACHE["nc"] = _build_module()
    return _MODULE_CACHE["nc"]


def kernel(x, x_len, w1, w2):
    global LAST_RESULTS
    from concourse.bass_utils import run_bass_kernel_spmd

    x = np.ascontiguousarray(np.asarray(x, dtype=np.float32))
    w1q = _fake_quant(np.asarray(w1, dtype=np.float32))  # [512, 80, 1]
    w2q = _fake_quant(np.asarray(w2, dtype=np.float32))  # [512, 1, 8]

    # wt[c, j, o] = w1q[o, c] * w2q[o, j]
    wt = np.ascontiguousarray(
        w1q[:, :, 0].T[:, None, :] * w2q[:, 0, :].T[None, :, :]
    ).astype(np.float32)  # [80, 8, 512]

    nc = _get_module()
    in_maps = [
        {"xs": x[i * BPC:(i + 1) * BPC], "wt": wt} for i in range(N_CORES)
    ]
    res = run_bass_kernel_spmd(nc, in_maps, core_ids=list(range(N_CORES)))
    LAST_RESULTS = res
    y = np.concatenate([r["out"] for r in res.results], axis=0)

    x_len = np.asarray(x_len)
    x_len_out = np.floor(x_len.astype(np.float32) / np.float32(SF)).astype(np.float32)
    return y, x_len_out


# revision 7
# speedup vs baseline: 1.0815x; 1.0815x over previous
"""Trainium2 Bass kernel for nn_Conv1dSubsampling (pointwise conv 80->512 +
depthwise conv k=8 stride=4, both with per-out-channel symmetric fake-quant).

Strategy:
  - Host: fake-quantize w1/w2, fuse into per-tap weights
      W[j, c, o] = w1q[o, c] * w2q[o, j]   (j = 0..7 tap, c in-ch, o out-ch)
    so that  out[b, o, u] = sum_j sum_c W[j, c, o] * x[b, c, 4u + j].
  - Device (8 cores, data-parallel over batch, 2 batches/core):
    for each output tile [128 out-ch, <=512 u]: accumulate 8 matmuls
    (K=80 contraction over in-ch, one per tap) into one PSUM bank.
    The per-tap rhs is a stride-4 view of the x tile (no im2col copy).
    Matmuls run in float32r (full PE rate); accumulation is fp32 in PSUM.
  - x_len_out = floor(x_len / 4) is computed on host (16 scalars).
"""

import numpy as np

IN_CH, OUT_CH, SF = 80, 512, 4
T_IN = 4096
BATCH = 16
N_CORES = 8
BPC = BATCH // N_CORES          # batches per core
KT = 2 * SF                     # depthwise taps
T_OUT = (T_IN - KT) // SF + 1   # 1023
QMAX = 2.0 ** 7 - 1.0
EPS = 1e-5

_MODULE_CACHE = {}
LAST_RESULTS = None  # BassKernelResults of the most recent run (for test.py)


def _fake_quant(w):
    """Match reference.fake_quant_weight: symmetric per-out-channel max quant."""
    w = w.astype(np.float32)
    s = np.max(np.abs(w.reshape(w.shape[0], -1)), axis=1) / np.float32(QMAX)
    s = np.maximum(s, np.float32(EPS)).astype(np.float32)
    sb = s.reshape((-1,) + (1,) * (w.ndim - 1))
    return (np.round(w / sb) * sb).astype(np.float32)


def _build_module():
    import concourse.bass as bass
    import concourse.tile as tile
    from concourse import bacc, mybir

    f32 = mybir.dt.float32
    f16 = mybir.dt.float16

    nc = bacc.Bacc(
        "TRN2",
        target_bir_lowering=False,
        debug=False,
        enable_asserts=False,
        num_devices=N_CORES,
    )
    xs_d = nc.dram_tensor("xs", [BPC, IN_CH, T_IN], f16, kind="ExternalInput").ap()
    wt_d = nc.dram_tensor("wt", [IN_CH, KT, OUT_CH], f16, kind="ExternalInput").ap()
    out_d = nc.dram_tensor("out", [BPC, OUT_CH, T_OUT], f32, kind="ExternalOutput").ap()

    M_TILES = OUT_CH // 128     # 4
    U_BLK = 512

    with tile.TileContext(nc) as tc:
        with (
            tc.tile_pool(name="wp", bufs=1) as wp,
            tc.tile_pool(name="xp", bufs=2) as xp,
            tc.tile_pool(name="op", bufs=3) as op,
            tc.tile_pool(name="ps", bufs=4, space=bass.MemorySpace.PSUM) as ps,
        ):
            w_sb = wp.tile([IN_CH, KT, OUT_CH], f16)
            nc.scalar.dma_start(out=w_sb, in_=wt_d)

            for b in range(BPC):
                x_sb = xp.tile([IN_CH, T_IN], f16)
                nc.sync.dma_start(out=x_sb, in_=xs_d[b])
                xv = x_sb.rearrange("c (u r) -> c u r", r=SF)  # [80, 1024, 4]

                for m in range(M_TILES):
                    o_sb = op.tile([128, T_OUT], f32)
                    # fp32r matmuls need an even free dim; cover 1023 outputs
                    # with two overlapping 512-wide blocks (u=511 done twice).
                    for u0 in (0, T_OUT - U_BLK):
                        n = U_BLK
                        acc = ps.tile([128, U_BLK], f32)
                        for j in range(KT):
                            # rhs: x[b, c, 4u + j] for u in [u0, u0+n)
                            q, r = divmod(j, SF)
                            rhs = xv[:, u0 + q:u0 + q + n, r]
                            nc.tensor.matmul(
                                acc[:, :n],
                                w_sb[:, j, m * 128:(m + 1) * 128],
                                rhs,
                                start=(j == 0),
                                stop=(j == KT - 1),
                            )
                        # Spread PSUM->SBUF copies over DVE and ACT
                        if m % 2 == 0:
                            nc.vector.tensor_copy(o_sb[:, u0:u0 + n], acc[:, :n])
                        else:
                            nc.scalar.copy(o_sb[:, u0:u0 + n], acc[:, :n])
                    nc.sync.dma_start(
                        out=out_d[b, m * 128:(m + 1) * 128, :], in_=o_sb
                    )
    nc.compile()
    return nc


def _get_module():
    if "nc" not in _MODULE_CACHE:
        _MODULE_CACHE["nc"] = _build_module()
    return _MODULE_CACHE["nc"]


def kernel(x, x_len, w1, w2):
    global LAST_RESULTS
    from concourse.bass_utils import run_bass_kernel_spmd

    x = np.ascontiguousarray(np.asarray(x, dtype=np.float32))
    w1q = _fake_quant(np.asarray(w1, dtype=np.float32))  # [512, 80, 1]
    w2q = _fake_quant(np.asarray(w2, dtype=np.float32))  # [512, 1, 8]

    # wt[c, j, o] = w1q[o, c] * w2q[o, j]
    wt = np.ascontiguousarray(
        w1q[:, :, 0].T[:, None, :] * w2q[:, 0, :].T[None, :, :]
    ).astype(np.float32)  # [80, 8, 512]

    nc = _get_module()
    x16 = x.astype(np.float16)
    wt16 = wt.astype(np.float16)
    in_maps = [
        {"xs": x16[i * BPC:(i + 1) * BPC], "wt": wt16} for i in range(N_CORES)
    ]
    res = run_bass_kernel_spmd(nc, in_maps, core_ids=list(range(N_CORES)))
    LAST_RESULTS = res
    y = np.concatenate([r["out"] for r in res.results], axis=0)

    x_len = np.asarray(x_len)
    x_len_out = np.floor(x_len.astype(np.float32) / np.float32(SF)).astype(np.float32)
    return y, x_len_out


# revision 8
# speedup vs baseline: 2.0041x; 1.8530x over previous
"""Trainium2 Bass kernel for nn_Conv1dSubsampling (pointwise conv 80->512 +
depthwise conv k=8 stride=4, both with per-out-channel symmetric fake-quant).

Formulation: the pointwise+depthwise pair is one strided conv with fused
weight W[o, c, j] = w1q[o, c] * w2q[o, j] (j = tap 0..7):

    out[b, o, u] = sum_{c,j} W[o, c, j] * x[b, c, 4u + j]       u = 0..1022

Split taps j = 4a + r (a in {0,1}, r in {0..3}) and define the phase-
decimated input z[(r*80+c), u] = x[c, 4u + r] (a pure host-side reshape/
transpose of x). Then with A[(r,c), o] = W[o, c, r], B[(r,c), o] = W[o, c, 4+r]:

    out[:, u] = A^T z[:, u] + B^T z[:, u+1]

i.e. a K=640 GEMM against z at column offsets 0 and +1. K-tiles (host-packed):
    T0 = z rows   0..127        @u   (A rows   0..127)
    T1 = z rows 128..255        @u   (A rows 128..255)
    T2 = [z rows 256..319 @u ; z rows 256..319 @u+1]  (A|B rows 256..319)
    T0 = z rows   0..127        @u+1 (B rows   0..127)   <- same SBUF tile
    T1 = z rows 128..255        @u+1 (B rows 128..255)   <- same SBUF tile
T2's +1 shift is baked in on the host, so each output tile is exactly 5
accumulating fp16 matmuls into one PSUM bank (K=128/128/128/128/128).

Sharding: data-parallel over batch, 2 batches per core, 8 cores. The host
precomputes fake-quant + weight fusion + im2col packing (cheap numpy); the
device does all the FLOPs. x_len_out = floor(x_len/4) on host (16 scalars).
"""

import numpy as np

IN_CH, OUT_CH, SF = 80, 512, 4
T_IN = 4096
BATCH = 16
N_CORES = 8
BPC = BATCH // N_CORES          # batches per core
KT = 2 * SF                     # depthwise taps
T_OUT = (T_IN - KT) // SF + 1   # 1023
U = T_IN // SF                  # 1024 columns of z
KZ = SF * IN_CH                 # 320 rows of z (phase-major: k = r*80 + c)
QMAX = 2.0 ** 7 - 1.0
EPS = 1e-5

_MODULE_CACHE = {}
LAST_RESULTS = None  # BassKernelResults of the most recent run (for test.py)


def _fake_quant(w):
    """Match reference.fake_quant_weight: symmetric per-out-channel max quant."""
    w = w.astype(np.float32)
    s = np.max(np.abs(w.reshape(w.shape[0], -1)), axis=1) / np.float32(QMAX)
    s = np.maximum(s, np.float32(EPS)).astype(np.float32)
    sb = s.reshape((-1,) + (1,) * (w.ndim - 1))
    return (np.round(w / sb) * sb).astype(np.float32)


def _build_module():
    import concourse.bass as bass
    import concourse.tile as tile
    from concourse import bacc, mybir

    f32 = mybir.dt.float32
    f16 = mybir.dt.float16

    nc = bacc.Bacc(
        "TRN2",
        target_bir_lowering=False,
        debug=False,
        enable_asserts=False,
        num_devices=N_CORES,
    )
    # z: per batch 3 packed k-tile planes [128, 1024] (T0, T1, T2mix)
    zs_d = nc.dram_tensor("zs", [BPC, 3, 128, U], f16, kind="ExternalInput").ap()
    # weights: 5 k-tiles x [128, 512]
    wt_d = nc.dram_tensor("wt", [5, 128, OUT_CH], f16, kind="ExternalInput").ap()
    out_d = nc.dram_tensor("out", [BPC, OUT_CH, T_OUT], f32, kind="ExternalOutput").ap()

    M_TILES = OUT_CH // 128     # 4
    U_BLK = 512

    with tile.TileContext(nc) as tc:
        with (
            tc.tile_pool(name="wp", bufs=1) as wp,
            tc.tile_pool(name="zp", bufs=2) as zp,
            tc.tile_pool(name="op", bufs=3) as op,
            tc.tile_pool(name="ps", bufs=4, space=bass.MemorySpace.PSUM) as ps,
        ):
            # w_sb[p, kt, o]: lhsT for (kt, m) = w_sb[:, kt, 128m:128m+128]
            w_sb = wp.tile([128, 5, OUT_CH], f16)
            nc.scalar.dma_start(
                out=w_sb, in_=wt_d.rearrange("t p o -> p t o")
            )

            for b in range(BPC):
                # z_sb[p, t, u]: k-tile t = z_sb[:, t, :]
                z_sb = zp.tile([128, 3, U], f16)
                nc.sync.dma_start(
                    out=z_sb, in_=zs_d[b].rearrange("t p u -> p t u")
                )

                for m in range(M_TILES):
                    o_sb = op.tile([128, T_OUT], f32)
                    # even free dim needed; cover 1023 outputs with two
                    # overlapping 512-wide blocks (u=511 computed twice).
                    for u0 in (0, T_OUT - U_BLK):
                        acc = ps.tile([128, U_BLK], f32)
                        # (k-tile plane, z column offset, weight k-tile)
                        for i, (t, du, kt) in enumerate(
                            ((0, 0, 0), (1, 0, 1), (2, 0, 2), (0, 1, 3), (1, 1, 4))
                        ):
                            nc.tensor.matmul(
                                acc,
                                w_sb[:, kt, m * 128:(m + 1) * 128],
                                z_sb[:, t, u0 + du:u0 + du + U_BLK],
                                start=(i == 0),
                                stop=(i == 4),
                            )
                        # Spread PSUM->SBUF copies over DVE and ACT
                        if m % 2 == 0:
                            nc.vector.tensor_copy(o_sb[:, u0:u0 + U_BLK], acc)
                        else:
                            nc.scalar.copy(o_sb[:, u0:u0 + U_BLK], acc)
                    nc.sync.dma_start(
                        out=out_d[b, m * 128:(m + 1) * 128, :], in_=o_sb
                    )
    nc.compile()
    return nc


def _get_module():
    if "nc" not in _MODULE_CACHE:
        _MODULE_CACHE["nc"] = _build_module()
    return _MODULE_CACHE["nc"]


def _pack_inputs(x, w1, w2):
    """Host-side fake-quant, weight fusion, and im2col packing (all fp16)."""
    x = np.asarray(x, dtype=np.float32)
    w1q = _fake_quant(np.asarray(w1, dtype=np.float32))[:, :, 0]  # [512, 80]
    w2q = _fake_quant(np.asarray(w2, dtype=np.float32))[:, 0, :]  # [512, 8]

    # z[b, r*80+c, u] = x[b, c, 4u+r]  -> [B, 320, 1024]
    z = np.transpose(
        x.reshape(BATCH, IN_CH, U, SF), (0, 3, 1, 2)
    ).reshape(BATCH, KZ, U)
    z16 = z.astype(np.float16)

    # A[(r,c), o] = w1q[o,c]*w2q[o,r];  B[(r,c), o] = w1q[o,c]*w2q[o,4+r]
    wa = (w2q.T[:SF, None, :] * w1q.T[None, :, :]).reshape(KZ, OUT_CH)
    wb = (w2q.T[SF:, None, :] * w1q.T[None, :, :]).reshape(KZ, OUT_CH)

    # 5 weight k-tiles matching the z tiling
    wt = np.empty((5, 128, OUT_CH), np.float32)
    wt[0] = wa[0:128]
    wt[1] = wa[128:256]
    wt[2, 0:64] = wa[256:320]
    wt[2, 64:128] = wb[256:320]
    wt[3] = wb[0:128]
    wt[4] = wb[128:256]
    wt16 = np.ascontiguousarray(wt.astype(np.float16))

    # z k-tile planes per batch: T0, T1, T2mix ([:, :, u] with T2 rows 64:128
    # pre-shifted by one u so all 5 matmuls of a group share the column AP)
    zt = np.empty((BATCH, 3, 128, U), np.float16)
    zt[:, 0] = z16[:, 0:128]
    zt[:, 1] = z16[:, 128:256]
    zt[:, 2, 0:64, :] = z16[:, 256:320]
    zt[:, 2, 64:128, :U - 1] = z16[:, 256:320, 1:]
    zt[:, 2, 64:128, U - 1] = 0  # never read (u<=1022 uses col<=1023 via +0)
    return np.ascontiguousarray(zt), wt16


def kernel(x, x_len, w1, w2):
    global LAST_RESULTS
    from concourse.bass_utils import run_bass_kernel_spmd

    zt, wt16 = _pack_inputs(x, w1, w2)

    nc = _get_module()
    in_maps = [
        {"zs": zt[i * BPC:(i + 1) * BPC], "wt": wt16} for i in range(N_CORES)
    ]
    res = run_bass_kernel_spmd(nc, in_maps, core_ids=list(range(N_CORES)))
    LAST_RESULTS = res
    y = np.concatenate([r["out"] for r in res.results], axis=0)

    x_len = np.asarray(x_len)
    x_len_out = np.floor(x_len.astype(np.float32) / np.float32(SF)).astype(np.float32)
    return y, x_len_out


# revision 11
# speedup vs baseline: 2.2519x; 1.1237x over previous
"""Trainium2 Bass kernel for nn_Conv1dSubsampling (pointwise conv 80->512 +
depthwise conv k=8 stride=4, both with per-out-channel symmetric fake-quant).

Formulation: the pointwise+depthwise pair is one strided conv with fused
weight W[o, c, j] = w1q[o, c] * w2q[o, j] (j = tap 0..7):

    out[b, o, u] = sum_{c,j} W[o, c, j] * x[b, c, 4u + j]       u = 0..1022

Split taps j = 4a + r (a in {0,1}, r in {0..3}) and define the phase-
decimated input z[(r*80+c), u] = x[c, 4u + r] (a pure host-side reshape/
transpose of x). Then with A[(r,c), o] = W[o, c, r], B[(r,c), o] = W[o, c, 4+r]:

    out[:, u] = A^T z[:, u] + B^T z[:, u+1]

i.e. a K=640 GEMM against z at column offsets 0 and +1. K-tiles (host-packed):
    T0 = z rows   0..127        @u   (A rows   0..127)
    T1 = z rows 128..255        @u   (A rows 128..255)
    T2 = [z rows 256..319 @u ; z rows 256..319 @u+1]  (A|B rows 256..319)
    T0 = z rows   0..127        @u+1 (B rows   0..127)   <- same SBUF tile
    T1 = z rows 128..255        @u+1 (B rows 128..255)   <- same SBUF tile
T2's +1 shift is baked in on the host, so each output tile is exactly 5
accumulating fp16 matmuls into one PSUM bank (K=128/128/128/128/128).

Sharding: data-parallel over batch, 2 batches per core, 8 cores. The host
precomputes fake-quant + weight fusion + im2col packing (cheap numpy); the
device does all the FLOPs. x_len_out = floor(x_len/4) on host (16 scalars).
"""

import numpy as np

IN_CH, OUT_CH, SF = 80, 512, 4
T_IN = 4096
BATCH = 16
N_CORES = 8
BPC = BATCH // N_CORES          # batches per core
KT = 2 * SF                     # depthwise taps
T_OUT = (T_IN - KT) // SF + 1   # 1023
U = T_IN // SF                  # 1024 columns of z
KZ = SF * IN_CH                 # 320 rows of z (phase-major: k = r*80 + c)
QMAX = 2.0 ** 7 - 1.0
EPS = 1e-5

_MODULE_CACHE = {}
LAST_RESULTS = None  # BassKernelResults of the most recent run (for test.py)


def _fake_quant(w):
    """Match reference.fake_quant_weight: symmetric per-out-channel max quant."""
    w = w.astype(np.float32)
    s = np.max(np.abs(w.reshape(w.shape[0], -1)), axis=1) / np.float32(QMAX)
    s = np.maximum(s, np.float32(EPS)).astype(np.float32)
    sb = s.reshape((-1,) + (1,) * (w.ndim - 1))
    return (np.round(w / sb) * sb).astype(np.float32)


def _build_module():
    import concourse.bass as bass
    import concourse.tile as tile
    from concourse import bacc, mybir
    from concourse.vector_clock import ScopedClock

    f32 = mybir.dt.float32
    f16 = mybir.dt.float16

    class SlimTailTileContext(tile.TileContext):
        """TileContext whose exit path skips the trailing all-engine barrier.

        The stock tail is drain -> barrier -> sem clear -> barrier; the final
        barrier only matters when more code follows the TileContext, which is
        never the case here. Dropping it shaves several us off every run.
        """

        def _drain_and_barrier(self, tick_clock, wait_clock):
            drain_inst = self.nc.sync.drain()
            wait_clock.add_sem_waits(
                drain_inst.ins, ScopedClock({None: tick_clock.global_clock})
            )
            self.nc.all_engine_barrier()
            popped = self.nc._tile_sem_poison_stack.pop()
            assert popped is self._sem_poison
            self.nc.clear_and_free_semaphores(
                list(self.sems.allocated().values())
            )

    nc = bacc.Bacc(
        "TRN2",
        target_bir_lowering=False,
        debug=False,
        enable_asserts=False,
        num_devices=N_CORES,
    )
    # z: per batch 3 packed k-tile planes [128, 1024] (T0, T1, T2mix)
    zs_d = nc.dram_tensor("zs", [BPC, 3, 128, U], f16, kind="ExternalInput").ap()
    # weights: 5 k-tiles x [128, 512]
    wt_d = nc.dram_tensor("wt", [5, 128, OUT_CH], f16, kind="ExternalInput").ap()
    out_d = nc.dram_tensor("out", [BPC, OUT_CH, T_OUT], f32, kind="ExternalOutput").ap()

    M_TILES = OUT_CH // 128     # 4
    U_BLK = 512

    with SlimTailTileContext(nc) as tc:
        with (
            tc.tile_pool(name="wp", bufs=1) as wp,
            tc.tile_pool(name="zp", bufs=2) as zp,
            tc.tile_pool(name="op", bufs=3) as op,
            tc.tile_pool(name="ps", bufs=5, space=bass.MemorySpace.PSUM) as ps,
            tc.tile_pool(name="wps", bufs=1, space=bass.MemorySpace.PSUM) as wps,
        ):
            # Warm the PE clock (HAM ramp takes ~11 N=512 matmuls) during the
            # input-DMA wait: dummy matmuls on a zeroed tile into scratch PSUM.
            warm = wp.tile([128, 640], f16)
            nc.gpsimd.memset(warm, 0.0)
            wacc = wps.tile([128, U_BLK], f32)
            for i in range(14):
                nc.tensor.matmul(
                    wacc, warm[:, 0:128], warm[:, 128:640],
                    start=(i == 0), stop=(i == 13),
                )

            # w_sb[p, kt, o]: lhsT for (kt, m) = w_sb[:, kt, 128m:128m+128]
            w_sb = wp.tile([128, 5, OUT_CH], f16)
            nc.scalar.dma_start(
                out=w_sb, in_=wt_d.rearrange("t p o -> p t o")
            )

            for b in range(BPC):
                # z_sb[p, t, u]: k-tile t = z_sb[:, t, :]; one DMA per plane
                # (different queues) so the first matmul starts sooner.
                z_sb = zp.tile([128, 3, U], f16)
                for t, deng in ((0, nc.sync), (1, nc.gpsimd), (2, nc.sync)):
                    deng.dma_start(
                        out=z_sb[:, t, :], in_=zs_d[b, t]
                    )

                for m in range(M_TILES):
                    o_sb = op.tile([128, T_OUT], f32)
                    # even free dim needed; cover 1023 outputs with two
                    # overlapping 512-wide blocks (u=511 computed twice).
                    for u0 in (0, T_OUT - U_BLK):
                        acc = ps.tile([128, U_BLK], f32)
                        # (k-tile plane, z column offset, weight k-tile)
                        for i, (t, du, kt) in enumerate(
                            ((0, 0, 0), (1, 0, 1), (2, 0, 2), (0, 1, 3), (1, 1, 4))
                        ):
                            nc.tensor.matmul(
                                acc,
                                w_sb[:, kt, m * 128:(m + 1) * 128],
                                z_sb[:, t, u0 + du:u0 + du + U_BLK],
                                start=(i == 0),
                                stop=(i == 4),
                            )
                        # Spread PSUM->SBUF copies over DVE and ACT
                        if m % 2 == 0:
                            nc.vector.tensor_copy(o_sb[:, u0:u0 + U_BLK], acc)
                        else:
                            nc.scalar.copy(o_sb[:, u0:u0 + U_BLK], acc)
                    nc.sync.dma_start(
                        out=out_d[b, m * 128:(m + 1) * 128, :], in_=o_sb
                    )
    nc.compile()
    return nc


def _get_module():
    if "nc" not in _MODULE_CACHE:
        _MODULE_CACHE["nc"] = _build_module()
    return _MODULE_CACHE["nc"]


def _pack_inputs(x, w1, w2):
    """Host-side fake-quant, weight fusion, and im2col packing (all fp16)."""
    x = np.asarray(x, dtype=np.float32)
    w1q = _fake_quant(np.asarray(w1, dtype=np.float32))[:, :, 0]  # [512, 80]
    w2q = _fake_quant(np.asarray(w2, dtype=np.float32))[:, 0, :]  # [512, 8]

    # z[b, r*80+c, u] = x[b, c, 4u+r]  -> [B, 320, 1024]
    z = np.transpose(
        x.reshape(BATCH, IN_CH, U, SF), (0, 3, 1, 2)
    ).reshape(BATCH, KZ, U)
    z16 = z.astype(np.float16)

    # A[(r,c), o] = w1q[o,c]*w2q[o,r];  B[(r,c), o] = w1q[o,c]*w2q[o,4+r]
    wa = (w2q.T[:SF, None, :] * w1q.T[None, :, :]).reshape(KZ, OUT_CH)
    wb = (w2q.T[SF:, None, :] * w1q.T[None, :, :]).reshape(KZ, OUT_CH)

    # 5 weight k-tiles matching the z tiling
    wt = np.empty((5, 128, OUT_CH), np.float32)
    wt[0] = wa[0:128]
    wt[1] = wa[128:256]
    wt[2, 0:64] = wa[256:320]
    wt[2, 64:128] = wb[256:320]
    wt[3] = wb[0:128]
    wt[4] = wb[128:256]
    wt16 = np.ascontiguousarray(wt.astype(np.float16))

    # z k-tile planes per batch: T0, T1, T2mix ([:, :, u] with T2 rows 64:128
    # pre-shifted by one u so all 5 matmuls of a group share the column AP)
    zt = np.empty((BATCH, 3, 128, U), np.float16)
    zt[:, 0] = z16[:, 0:128]
    zt[:, 1] = z16[:, 128:256]
    zt[:, 2, 0:64, :] = z16[:, 256:320]
    zt[:, 2, 64:128, :U - 1] = z16[:, 256:320, 1:]
    zt[:, 2, 64:128, U - 1] = 0  # never read (u<=1022 uses col<=1023 via +0)
    return np.ascontiguousarray(zt), wt16


def kernel(x, x_len, w1, w2):
    global LAST_RESULTS
    from concourse.bass_utils import run_bass_kernel_spmd

    zt, wt16 = _pack_inputs(x, w1, w2)

    nc = _get_module()
    in_maps = [
        {"zs": zt[i * BPC:(i + 1) * BPC], "wt": wt16} for i in range(N_CORES)
    ]
    res = run_bass_kernel_spmd(nc, in_maps, core_ids=list(range(N_CORES)))
    LAST_RESULTS = res
    y = np.concatenate([r["out"] for r in res.results], axis=0)

    x_len = np.asarray(x_len)
    x_len_out = np.floor(x_len.astype(np.float32) / np.float32(SF)).astype(np.float32)
    return y, x_len_out
